# revision 38
# baseline (speedup 1.0000x reference)
"""DigitalMapper kernel for 8 trn2 NeuronCores.

Math: reference computes  out = (x @ softmax(W, axis=1).T) > 0.5  with
x in {0,1}.  Let E = exp(W) (row-unnormalized).  Then

  out[b,o] > 0.5  <=>  sum_i (2*x[b,i] - 1) * E[o,i] > 0

so softmax divide, row-max subtraction and the threshold fold into a
zero-threshold on a +-1 matmul against E (scaling a column by exp(m_o)>0
never changes sign; |W| <= ~5.5 so exp stays in fp32/fp8e4 range).

Precision scheme (all-fp8 PE): E is split into three e4m3 levels at
true scale
    H = rnd8(E),  M = rnd8(E - H),  L = rnd8(E - H - M)
giving ~12 mantissa bits (e4m3 denormals reach 2^-9, so residuals
quantize without explicit rescaling).  x side is +-1, exact in e4m3.
All three passes run as DoubleRow matmuls (0.5 cyc/row on the PE), so a
full K=2048 pass costs 32768 cycles and the kernel needs 3*32768 =
98304 PE cycles/core vs 163840 for the fp32r+fp8 baseline.  Host-model
flip count: 555/8.4M outputs (rel err 1.1e-2, gate 2e-2).

Sharding: BG=2 batch groups x OG=4 out-feature groups.  Each core gets
x.T[:, bg*2048:...] and W.T[:, og*512:...] and produces a [2048, 512]
block.  This shape minimizes per-core elementwise prep
(4 W-side passes * O_PER + 1 x-side pass * B_PER) so ACT/DVE/GPSIMD
keep pace with the PE.

Schedule: 16 m-tiles need 16 psum banks but only 8 exist, so two
phases of 8 m-tiles.  Phase 1 streams W+x-half0 per k-pair (768KB/pair
~= PE pace), phase 2 is PE-dense on resident tiles while x-half1 DMAs
and casts trail in.
"""

import sys

sys.path.insert(0, "/opt/trn_rl_repo")

import numpy as np

BATCH, IN_F, OUT_F = 4096, 2048, 2048
N_CORES = 8
BG, OG = 2, 4  # batch groups x out-feature groups
B_PER = BATCH // BG  # 2048 batch rows per core
O_PER = OUT_F // OG  # 512 out features per core
P = 128
KT = IN_F // P  # 16 contraction tiles
PAIRS = KT // 2  # 8 DoubleRow k-pairs
MT = B_PER // P  # 16 output row tiles per core
MGRP = 8  # m-tiles per psum phase (8 banks)
NFREE = O_PER  # matmul moving free dim = 512 (one psum bank)
BH = B_PER // 2  # deferred x half
N_WARM_MM = 80  # dummy matmuls that burn the PE p-state ramp

_COMPILED = {}


def _patch_tile_drain():
    """walrus in this container allows only ONE sem-wait per CTRL (Drain/NOP)
    instruction; Tile's kernel-tail drain aggregates one wait per live
    semaphore.  Split the waits across a chain of SP nops."""
    import concourse.mybir as mybir
    import concourse.tile as tile_mod
    from concourse.vector_clock import ScopedClock

    if getattr(tile_mod.TileContext, "_drain_split_patched", False):
        return

    def _drain_and_barrier_split(self, tick_clock, wait_clock):
        nc = self.nc
        drain_inst = nc.sync.drain()
        wait_clock.add_sem_waits(
            drain_inst.ins, ScopedClock({None: tick_clock.global_clock})
        )
        si = drain_inst.ins.sync_info
        waits = list(si.on_wait) if si is not None else []
        if len(waits) > 1:
            si.on_wait.clear()
            si.on_wait.extend(waits[:1])
            for w in waits[1:]:
                nop = nc.sync.nop(nofuse=True)
                if nop.ins.sync_info is None:
                    nop.ins.sync_info = mybir.SyncInfo(on_wait=[], on_update=[])
                nop.ins.sync_info.on_wait.append(w)
        nc.all_engine_barrier()
        assert self.sems is not None
        popped = nc._tile_sem_poison_stack.pop()
        assert popped is self._sem_poison
        nc.clear_and_free_semaphores(list(self.sems.allocated().values()))
        nc.all_engine_barrier()

    tile_mod.TileContext._drain_and_barrier = _drain_and_barrier_split
    tile_mod.TileContext._drain_split_patched = True


def _split_multi_waits(nc):
    """walrus here allows very few sem-waits per instruction.  Hoist extra
    waits onto same-engine NOPs placed immediately before the instruction
    (same blocking point, engine executes in order).  DMA-queue instructions
    keep their waits - their sync runs through the DGE queues."""
    import concourse.mybir as mybir

    n = 0
    for f in nc.m.functions:
        for bb in f.blocks:
            new_insts = []
            for inst in bb.instructions:
                si = inst.sync_info
                if si is not None and si.on_wait and len(si.on_wait) > 1:
                    waits = list(si.on_wait)
                    si.on_wait.clear()
                    si.on_wait.append(waits[0])
                    for w in waits[1:]:
                        n += 1
                        new_insts.append(
                            mybir.InstNoOp(
                                name=f"wsplit-{n}",
                                opcode="NoOp",
                                engine=inst.engine,
                                sync_info=mybir.SyncInfo(on_wait=[w], on_update=[]),
                                bass_nofuse=True,
                            )
                        )
                new_insts.append(inst)
            if n:
                try:
                    bb.instructions[:] = new_insts
                except TypeError:
                    bb.instructions = new_insts
    return n


def _build(split_waits: bool = True):
    """One core's SPMD program: 3-level e4m3 DoubleRow matmul."""
    import concourse.bass as bass
    import concourse.mybir as mybir
    import concourse.tile as tile

    _patch_tile_drain()

    f32 = mybir.dt.float32
    fp8 = mybir.dt.float8e4
    u8 = mybir.dt.uint8
    Alu = mybir.AluOpType
    Act = mybir.ActivationFunctionType
    DR = mybir.MatmulPerfMode.DoubleRow

    nc = bass.Bass()
    xt = nc.dram_tensor("xt", [IN_F, B_PER], u8, kind="ExternalInput")
    wt = nc.dram_tensor("wt", [IN_F, O_PER], f32, kind="ExternalInput")
    out = nc.dram_tensor("out", [B_PER, O_PER], u8, kind="ExternalOutput")

    # activation(bias=-1.0) needs a registered const AP
    _cm1 = nc.alloc_sbuf_tensor("const-float32-m1", [P, 1], f32)
    nc.gpsimd.memset(_cm1.ap(), -1.0)
    nc.const_aps.aps[(f32, -1.0)] = _cm1.ap()
    nc.all_engine_barrier()

    # xb engine split (phase-1 casts cover x columns [0, BH))
    XB1_DVE_A = slice(0, 350)
    XB1_ACT = slice(350, 900)
    XB1_DVE_B = slice(900, BH)

    with tile.TileContext(nc) as tc:
        with (
            tc.tile_pool(name="xu", bufs=1) as xu_pool,
            tc.tile_pool(name="wr", bufs=4) as wr_pool,
            tc.tile_pool(name="tr", bufs=2) as tr_pool,
            tc.tile_pool(name="h8", bufs=1) as h8_pool,
            tc.tile_pool(name="m8", bufs=1) as m8_pool,
            tc.tile_pool(name="l8", bufs=1) as l8_pool,
            tc.tile_pool(name="xb", bufs=1) as xb_pool,
            tc.tile_pool(name="ps", bufs=1, space="PSUM") as ps_pool,
            tc.tile_pool(name="ot", bufs=4) as ot_pool,
        ):
            warm = wr_pool.tile([P, 1], f32, name="warm", tag="warm")
            nc.vector.memset(warm[:], 0.0)

            xu, h8, m8, l8, xb = [], [], [], [], []
            wr_tiles = {}

            # PE p-state warmup: the cost model runs the PE at 2x cycle time
            # for its first 3us of continuous execution.  Burn that ramp on
            # tiny dummy matmuls (into a region the real phase-1 chain later
            # restarts with start=True) while the first DMAs are in flight,
            # so every real matmul runs at full rate.
            dumm = xb_pool.tile([P, 64], fp8, name="dumm", tag="dumm")
            nc.vector.memset(dumm[:], 0.0)
            warm_ps = ps_pool.tile([P, NFREE], f32, name="pn0", tag="pn0")
            for _ in range(N_WARM_MM):
                nc.tensor.matmul(
                    warm_ps[0:64, 0:64], dumm[:, 0:64], dumm[:, 0:64],
                    start=True, stop=True,
                )

            def dma_k(k):
                # The cost model charges each DMA's transfer time to the
                # issuing engine (500ns floor), so DMA issue is load-balanced
                # like compute: W + x-half0 on the otherwise-idle SP queue
                # (paces phase 1 at ~2.6us/pair), x-half1 on Pool and out on
                # SP/ACT during the PE-dense phase 2.  The first pair's four
                # DMAs spread across SP/ACT/Pool so they land concurrently.
                xk = xu_pool.tile([P, B_PER], u8, name=f"xu{k}", tag=f"xu{k}")
                xu.append(xk)
                wr = wr_pool.tile([P, O_PER], f32, name="wr", tag="wr")
                wr_tiles[k] = wr
                weng = nc.scalar if k == 1 else nc.sync
                # first pair: half-tile DMAs so the prep chain starts sooner
                for sl in ([slice(0, 256), slice(256, O_PER)] if k < 2
                           else [slice(0, O_PER)]):
                    weng.dma_start(wr[:, sl], wt[k * P : (k + 1) * P, sl])
                # phase 1 only needs x columns [0, BH); issue on ACT so SP
                # carries only the W stream (both stay under the PE pace)
                xeng = nc.gpsimd if k < 2 else nc.scalar
                xeng.dma_start(xk[:, 0:BH], xt[k * P : (k + 1) * P, 0:BH])

            def alloc_pair(p):
                h8.append(
                    h8_pool.tile([P, 2, O_PER], fp8, name=f"h{p}", tag=f"h{p}")
                )
                m8.append(
                    m8_pool.tile([P, 2, O_PER], fp8, name=f"m{p}", tag=f"m{p}")
                )
                l8.append(
                    l8_pool.tile([P, 2, O_PER], fp8, name=f"l{p}", tag=f"l{p}")
                )
                xb.append(
                    xb_pool.tile([P, 2, B_PER], fp8, name=f"x{p}", tag=f"x{p}")
                )

            def prep_pair0():
                # startup-critical pair: emit stage-interleaved (k0/k1
                # alternating per stage) in half-tile slices so no engine
                # queue holds back the H -> M -> L readiness ladder
                alloc_pair(0)
                trs = [
                    tr_pool.tile([P, O_PER], f32, name="tr", tag="tr")
                    for _ in range(2)
                ]
                S = [slice(0, 256), slice(256, O_PER)]
                for k in (0, 1):
                    for sl in S:
                        nc.scalar.activation(
                            wr_tiles[k][:, sl], wr_tiles[k][:, sl], Act.Exp
                        )
                for k in (0, 1):
                    for sl in S:
                        nc.gpsimd.tensor_copy(h8[0][:, k, sl], wr_tiles[k][:, sl])
                # xb in 512-col chunks: the first m-tiles only gate on the
                # first chunk, so the H matmuls start ~1us sooner
                for xc in (slice(0, 512), slice(512, BH)):
                    for k in (0, 1):
                        nc.vector.tensor_scalar(
                            xb[0][:, k, xc], xu[k][:, xc],
                            2.0, 1.0, Alu.mult, Alu.subtract,
                        )
                # k0's M/tr/L ladder on GPS, k1's on DVE (behind xb), so both
                # finish just ahead of the M- and L-matmul blocks
                for sl in S:
                    nc.gpsimd.tensor_tensor(
                        m8[0][:, 0, sl], wr_tiles[0][:, sl],
                        h8[0][:, 0, sl], Alu.subtract,
                    )
                    nc.gpsimd.tensor_tensor(
                        trs[0][:, sl], wr_tiles[0][:, sl],
                        h8[0][:, 0, sl], Alu.subtract,
                    )
                for sl in S:
                    nc.vector.tensor_tensor(
                        m8[0][:, 1, sl], wr_tiles[1][:, sl],
                        h8[0][:, 1, sl], Alu.subtract,
                    )
                    nc.vector.tensor_tensor(
                        trs[1][:, sl], wr_tiles[1][:, sl],
                        h8[0][:, 1, sl], Alu.subtract,
                    )
                for sl in S:
                    nc.gpsimd.tensor_tensor(
                        l8[0][:, 0, sl], trs[0][:, sl], m8[0][:, 0, sl],
                        Alu.subtract,
                    )
                for sl in S:
                    nc.vector.tensor_tensor(
                        l8[0][:, 1, sl], trs[1][:, sl], m8[0][:, 1, sl],
                        Alu.subtract,
                    )

            def prep_k(k):
                p, j = divmod(k, 2)
                xk = xu[k]
                if j == 0:
                    alloc_pair(p)
                wr = wr_tiles[k]
                tr = tr_pool.tile([P, O_PER], f32, name="tr", tag="tr")
                # E = exp(W); H/M/L = successive e4m3 roundings.
                # Engine balance (per k, PE pace is ~1284ns/k):
                #   ACT exp+xb1-mid ~1255, GPS H/tr/L ~1281, DVE M+xb1 ~1020
                nc.scalar.activation(wr[:], wr[:], Act.Exp)
                nc.gpsimd.tensor_copy(h8[p][:, j, :], wr[:])
                nc.vector.tensor_tensor(
                    m8[p][:, j, :], wr[:], h8[p][:, j, :], Alu.subtract
                )
                nc.gpsimd.tensor_tensor(
                    tr[:], wr[:], h8[p][:, j, :], Alu.subtract
                )
                nc.gpsimd.tensor_tensor(
                    l8[p][:, j, :], tr[:], m8[p][:, j, :], Alu.subtract
                )
                # xb = 2x-1 in {-1,+1}, exact in e4m3; phase-1 half, on DVE
                nc.vector.tensor_scalar(
                    xb[p][:, j, 0:BH], xk[:, 0:BH],
                    2.0, 1.0, Alu.mult, Alu.subtract,
                )

            def mm3(p, m, ps):
                for lvl, tt in enumerate((h8[p], m8[p], l8[p])):
                    nc.tensor.matmul(
                        ps[:],
                        xb[p][:, :, m * P : (m + 1) * P],
                        tt[:, :, :],
                        start=(p == 0 and lvl == 0),
                        stop=(p == PAIRS - 1 and lvl == 2),
                        perf_mode=DR,
                    )

            def evict(m, ps, eng=None):
                otm = ot_pool.tile([P, NFREE], u8, name="otm", tag="otm")
                nc.vector.tensor_scalar(otm[:], ps[:], 0.0, None, Alu.is_gt)
                # phase 1 stores go on SP (ACT is busy with xb2 casts then);
                # phase 2 alternates queues so the tail isn't serialized
                if eng is None:
                    eng = nc.sync if m % 2 == 0 else nc.scalar
                eng.dma_start(out[m * P : (m + 1) * P, :], otm[:])

            # ---- phase 1: m 0..8, k-streaming order ----
            # DMAs run one pair ahead of prep so the Pool queue's in-order
            # compute waits never delay the next pair's transfers.
            pss = {
                m: ps_pool.tile([P, NFREE], f32, name=f"pn{m % MGRP}",
                                tag=f"pn{m % MGRP}")
                for m in range(MGRP)
            }
            dma_k(0)
            dma_k(1)
            # warm the Exp table only after the k1 DMAs are queued on ACT —
            # the 1.3us table load must not delay those issues
            nc.scalar.activation(warm[:], warm[:], Act.Exp)
            for p in range(PAIRS):
                if p + 1 < PAIRS:
                    dma_k(2 * p + 2)
                    dma_k(2 * p + 3)
                if p == 0:
                    prep_pair0()
                else:
                    prep_k(2 * p)
                    prep_k(2 * p + 1)
                # level-outer: the pair's H matmuls only need the H tiles,
                # giving the M/L prep ops ~0.9us of PE time to land
                for lvl, tt in enumerate((h8[p], m8[p], l8[p])):
                    for m in range(MGRP):
                        nc.tensor.matmul(
                            pss[m][:],
                            xb[p][:, :, m * P : (m + 1) * P],
                            tt[:, :, :],
                            start=(p == 0 and lvl == 0),
                            stop=(p == PAIRS - 1 and lvl == 2),
                            perf_mode=DR,
                        )
            # deferred x halves: DMA + casts trail into phase 2.  Emitted
            # BEFORE the phase-1 evicts so the first casts aren't queued
            # behind evict work on DVE/SP; k0/k1 go on ACT which frees first.
            for k in range(KT):
                nc.sync.dma_start(
                    xu[k][:, BH:B_PER], xt[k * P : (k + 1) * P, BH:B_PER]
                )
            def xb2(k):
                p, j = divmod(k, 2)
                # ACT takes even k, GPS odd — DVE stays free for evicts
                if k % 2 == 0:
                    nc.scalar.activation(
                        xb[p][:, j, BH:B_PER], xu[k][:, BH:B_PER],
                        Act.Identity, bias=-1.0, scale=2.0,
                    )
                else:
                    nc.gpsimd.tensor_scalar(
                        xb[p][:, j, BH:B_PER], xu[k][:, BH:B_PER],
                        2.0, 1.0, Alu.mult, Alu.subtract,
                    )

            for k in range(KT):
                xb2(k)
            for m in range(MGRP):
                evict(m, pss[m], eng=nc.sync)

            # ---- phase 2: m 8..16, PE-dense on resident tiles ----
            # m-subgroups so psum chains stop (and evict+store) staggered
            # through the phase instead of piling up at the kernel tail
            pss2 = {
                m: ps_pool.tile([P, NFREE], f32, name=f"pn{m % MGRP}",
                                tag=f"pn{m % MGRP}")
                for m in range(MGRP, MT)
            }
            for sub in ((8, 12), (12, 15), (15, 16)):
                for p in range(PAIRS):
                    for m in range(*sub):
                        mm3(p, m, pss2[m])
                        if p == PAIRS - 1:
                            evict(m, pss2[m])

    if split_waits:
        _split_multi_waits(nc)
    return nc


def _get_compiled():
    if "fp83" not in _COMPILED:
        _COMPILED["fp83"] = _build()
    return _COMPILED["fp83"]


def kernel(x: np.ndarray, raw_weight: np.ndarray, _trace: bool = False):
    from concourse.bass_utils import run_bass_kernel_spmd

    nc = _get_compiled()

    x = np.asarray(x)
    raw_weight = np.asarray(raw_weight)

    # x is exactly 0.0/1.0; uint8 encodes it losslessly and quarters the DMA
    xT = np.ascontiguousarray(x.T.astype(np.uint8))
    wT = np.ascontiguousarray(raw_weight.T).astype(np.float32, copy=False)

    in_maps = []
    for c in range(N_CORES):
        bg, og = divmod(c, OG)
        in_maps.append(
            {
                "xt": np.ascontiguousarray(xT[:, bg * B_PER : (bg + 1) * B_PER]),
                "wt": np.ascontiguousarray(wT[:, og * O_PER : (og + 1) * O_PER]),
            }
        )

    res = run_bass_kernel_spmd(
        nc, in_maps, core_ids=list(range(N_CORES)), trace=_trace
    )

    full = np.empty((BATCH, OUT_F), dtype=x.dtype)
    for c in range(N_CORES):
        bg, og = divmod(c, OG)
        full[bg * B_PER : (bg + 1) * B_PER, og * O_PER : (og + 1) * O_PER] = (
            res.results[c]["out"]
        )
    if _trace:
        kernel.last_results = res
    return full


# revision 42
# speedup vs baseline: 1.0790x; 1.0790x over previous
"""DigitalMapper kernel for 8 trn2 NeuronCores.

Math: reference computes  out = (x @ softmax(W, axis=1).T) > 0.5  with
x in {0,1}.  Let E = exp(W) (row-unnormalized).  Then

  out[b,o] > 0.5  <=>  sum_i (2*x[b,i] - 1) * E[o,i] > 0

so softmax divide, row-max subtraction and the threshold fold into a
zero-threshold on a +-1 matmul against E (scaling a column by exp(m_o)>0
never changes sign; |W| <= ~5.5 so exp stays in fp32/fp8e4 range).

Precision scheme (all-fp8 PE): E is split into three e4m3 levels at
true scale
    H = rnd8(E),  M = rnd8(E - H),  L = rnd8(E - H - M)
giving ~12 mantissa bits (e4m3 denormals reach 2^-9, so residuals
quantize without explicit rescaling).  x side is +-1, exact in e4m3.
All three passes run as DoubleRow matmuls (0.5 cyc/row on the PE), so a
full K=2048 pass costs 32768 cycles and the kernel needs 3*32768 =
98304 PE cycles/core vs 163840 for the fp32r+fp8 baseline.  Host-model
flip count: 555/8.4M outputs (rel err 1.1e-2, gate 2e-2).

Sharding: BG=2 batch groups x OG=4 out-feature groups.  Each core gets
x.T[:, bg*2048:...] and W.T[:, og*512:...] and produces a [2048, 512]
block.  This shape minimizes per-core elementwise prep
(4 W-side passes * O_PER + 1 x-side pass * B_PER) so ACT/DVE/GPSIMD
keep pace with the PE.

Schedule: 16 m-tiles need 16 psum banks but only 8 exist, so two
phases of 8 m-tiles.  Phase 1 streams W+x-half0 per k-pair (768KB/pair
~= PE pace), phase 2 is PE-dense on resident tiles while x-half1 DMAs
and casts trail in.
"""

import sys

sys.path.insert(0, "/opt/trn_rl_repo")

import numpy as np

BATCH, IN_F, OUT_F = 4096, 2048, 2048
N_CORES = 8
BG, OG = 2, 4  # batch groups x out-feature groups
B_PER = BATCH // BG  # 2048 batch rows per core
O_PER = OUT_F // OG  # 512 out features per core
P = 128
KT = IN_F // P  # 16 contraction tiles
PAIRS = KT // 2  # 8 DoubleRow k-pairs
MT = B_PER // P  # 16 output row tiles per core
MGRP = 8  # m-tiles per psum phase (8 banks)
NFREE = O_PER  # matmul moving free dim = 512 (one psum bank)
BH = B_PER // 2  # deferred x half
N_WARM_MM = 80  # dummy matmuls that burn the PE p-state ramp

_COMPILED = {}


def _patch_tile_drain():
    """walrus in this container allows only ONE sem-wait per CTRL (Drain/NOP)
    instruction; Tile's kernel-tail drain aggregates one wait per live
    semaphore.  Split the waits across a chain of SP nops."""
    import concourse.mybir as mybir
    import concourse.tile as tile_mod
    from concourse.vector_clock import ScopedClock

    if getattr(tile_mod.TileContext, "_drain_split_patched", False):
        return

    def _drain_and_barrier_split(self, tick_clock, wait_clock):
        nc = self.nc
        drain_inst = nc.sync.drain()
        wait_clock.add_sem_waits(
            drain_inst.ins, ScopedClock({None: tick_clock.global_clock})
        )
        si = drain_inst.ins.sync_info
        waits = list(si.on_wait) if si is not None else []
        if len(waits) > 1:
            si.on_wait.clear()
            si.on_wait.extend(waits[:1])
            for w in waits[1:]:
                nop = nc.sync.nop(nofuse=True)
                if nop.ins.sync_info is None:
                    nop.ins.sync_info = mybir.SyncInfo(on_wait=[], on_update=[])
                nop.ins.sync_info.on_wait.append(w)
        nc.all_engine_barrier()
        assert self.sems is not None
        popped = nc._tile_sem_poison_stack.pop()
        assert popped is self._sem_poison
        nc.clear_and_free_semaphores(list(self.sems.allocated().values()))
        nc.all_engine_barrier()

    tile_mod.TileContext._drain_and_barrier = _drain_and_barrier_split
    tile_mod.TileContext._drain_split_patched = True


def _split_multi_waits(nc):
    """walrus here allows very few sem-waits per instruction.  Hoist extra
    waits onto same-engine NOPs placed immediately before the instruction
    (same blocking point, engine executes in order).  DMA-queue instructions
    keep their waits - their sync runs through the DGE queues."""
    import concourse.mybir as mybir

    n = 0
    for f in nc.m.functions:
        for bb in f.blocks:
            new_insts = []
            for inst in bb.instructions:
                si = inst.sync_info
                if si is not None and si.on_wait and len(si.on_wait) > 1:
                    waits = list(si.on_wait)
                    si.on_wait.clear()
                    si.on_wait.append(waits[0])
                    for w in waits[1:]:
                        n += 1
                        new_insts.append(
                            mybir.InstNoOp(
                                name=f"wsplit-{n}",
                                opcode="NoOp",
                                engine=inst.engine,
                                sync_info=mybir.SyncInfo(on_wait=[w], on_update=[]),
                                bass_nofuse=True,
                            )
                        )
                new_insts.append(inst)
            if n:
                try:
                    bb.instructions[:] = new_insts
                except TypeError:
                    bb.instructions = new_insts
    return n


def _build(split_waits: bool = True, x0_act: bool = False,
           xb1_mode: str = "d", l8_gps: bool = True):
    """One core's SPMD program: 3-level e4m3 DoubleRow matmul.

    x0_act: issue phase-1 x DMAs from ACT (else SP).
    xb1_mode: engine split for the phase-1 xb cast — 'ag' ACT+GPS,
        'ad' ACT+DVE, 'd' all-DVE.
    l8_gps: L8 subtract on GPSIMD (else DVE).
    """
    import concourse.bass as bass
    import concourse.mybir as mybir
    import concourse.tile as tile

    _patch_tile_drain()

    f32 = mybir.dt.float32
    fp8 = mybir.dt.float8e4
    u8 = mybir.dt.uint8
    Alu = mybir.AluOpType
    Act = mybir.ActivationFunctionType
    DR = mybir.MatmulPerfMode.DoubleRow

    nc = bass.Bass()
    xt = nc.dram_tensor("xt", [IN_F, B_PER], u8, kind="ExternalInput")
    wt = nc.dram_tensor("wt", [IN_F, O_PER], f32, kind="ExternalInput")
    out = nc.dram_tensor("out", [B_PER, O_PER], u8, kind="ExternalOutput")

    # activation(bias=-1.0) needs a registered const AP
    _cm1 = nc.alloc_sbuf_tensor("const-float32-m1", [P, 1], f32)
    nc.gpsimd.memset(_cm1.ap(), -1.0)
    nc.const_aps.aps[(f32, -1.0)] = _cm1.ap()
    nc.all_engine_barrier()

    # xb engine split (phase-1 casts cover x columns [0, BH))
    XB1_DVE_A = slice(0, 350)
    XB1_ACT = slice(350, 900)
    XB1_DVE_B = slice(900, BH)

    with tile.TileContext(nc) as tc:
        with (
            tc.tile_pool(name="xu", bufs=1) as xu_pool,
            tc.tile_pool(name="wr", bufs=4) as wr_pool,
            tc.tile_pool(name="tr", bufs=2) as tr_pool,
            tc.tile_pool(name="h8", bufs=1) as h8_pool,
            tc.tile_pool(name="m8", bufs=1) as m8_pool,
            tc.tile_pool(name="l8", bufs=1) as l8_pool,
            tc.tile_pool(name="xb", bufs=1) as xb_pool,
            tc.tile_pool(name="ps", bufs=1, space="PSUM") as ps_pool,
            tc.tile_pool(name="ot", bufs=4) as ot_pool,
        ):
            warm = wr_pool.tile([P, 1], f32, name="warm", tag="warm")
            nc.vector.memset(warm[:], 0.0)

            xu, h8, m8, l8, xb = [], [], [], [], []
            wr_tiles = {}

            # PE p-state warmup: the cost model runs the PE at 2x cycle time
            # for its first 3us of continuous execution.  Burn that ramp on
            # tiny dummy matmuls (into a region the real phase-1 chain later
            # restarts with start=True) while the first DMAs are in flight,
            # so every real matmul runs at full rate.
            dumm = xb_pool.tile([P, 64], fp8, name="dumm", tag="dumm")
            nc.vector.memset(dumm[:], 0.0)
            warm_ps = ps_pool.tile([P, NFREE], f32, name="pn0", tag="pn0")
            for _ in range(N_WARM_MM):
                nc.tensor.matmul(
                    warm_ps[0:64, 0:64], dumm[:, 0:64], dumm[:, 0:64],
                    start=True, stop=True,
                )

            def dma_k(k):
                # The cost model charges each DMA's transfer time to the
                # issuing engine (500ns floor), so DMA issue is load-balanced
                # like compute: W + x-half0 on the otherwise-idle SP queue
                # (paces phase 1 at ~2.6us/pair), x-half1 on Pool and out on
                # SP/ACT during the PE-dense phase 2.  The first pair's four
                # DMAs spread across SP/ACT/Pool so they land concurrently.
                xk = xu_pool.tile([P, B_PER], u8, name=f"xu{k}", tag=f"xu{k}")
                xu.append(xk)
                wr = wr_pool.tile([P, O_PER], f32, name="wr", tag="wr")
                wr_tiles[k] = wr
                weng = nc.scalar if k == 1 else nc.sync
                # first pair: half-tile DMAs so the prep chain starts sooner
                for sl in ([slice(0, 256), slice(256, O_PER)] if k < 2
                           else [slice(0, O_PER)]):
                    weng.dma_start(wr[:, sl], wt[k * P : (k + 1) * P, sl])
                # phase 1 only needs x columns [0, BH)
                xeng = (nc.gpsimd if k < 2
                        else nc.scalar if x0_act else nc.sync)
                xeng.dma_start(xk[:, 0:BH], xt[k * P : (k + 1) * P, 0:BH])

            def alloc_pair(p):
                h8.append(
                    h8_pool.tile([P, 2, O_PER], fp8, name=f"h{p}", tag=f"h{p}")
                )
                m8.append(
                    m8_pool.tile([P, 2, O_PER], fp8, name=f"m{p}", tag=f"m{p}")
                )
                l8.append(
                    l8_pool.tile([P, 2, O_PER], fp8, name=f"l{p}", tag=f"l{p}")
                )
                xb.append(
                    xb_pool.tile([P, 2, B_PER], fp8, name=f"x{p}", tag=f"x{p}")
                )

            def prep_pair0():
                # startup-critical pair: emit stage-interleaved (k0/k1
                # alternating per stage) in half-tile slices so no engine
                # queue holds back the H -> M -> L readiness ladder
                alloc_pair(0)
                trs = [
                    tr_pool.tile([P, O_PER], f32, name="tr", tag="tr")
                    for _ in range(2)
                ]
                S = [slice(0, 256), slice(256, O_PER)]
                for k in (0, 1):
                    for sl in S:
                        nc.scalar.activation(
                            wr_tiles[k][:, sl], wr_tiles[k][:, sl], Act.Exp
                        )
                for k in (0, 1):
                    for sl in S:
                        nc.gpsimd.tensor_copy(h8[0][:, k, sl], wr_tiles[k][:, sl])
                # xb in 512-col chunks: the first m-tiles only gate on the
                # first chunk, so the H matmuls start ~1us sooner
                for xc in (slice(0, 512), slice(512, BH)):
                    for k in (0, 1):
                        nc.vector.tensor_scalar(
                            xb[0][:, k, xc], xu[k][:, xc],
                            2.0, 1.0, Alu.mult, Alu.subtract,
                        )
                # k0's M/tr/L ladder on GPS, k1's on DVE (behind xb), so both
                # finish just ahead of the M- and L-matmul blocks
                for sl in S:
                    nc.gpsimd.tensor_tensor(
                        m8[0][:, 0, sl], wr_tiles[0][:, sl],
                        h8[0][:, 0, sl], Alu.subtract,
                    )
                    nc.gpsimd.tensor_tensor(
                        trs[0][:, sl], wr_tiles[0][:, sl],
                        h8[0][:, 0, sl], Alu.subtract,
                    )
                for sl in S:
                    nc.vector.tensor_tensor(
                        m8[0][:, 1, sl], wr_tiles[1][:, sl],
                        h8[0][:, 1, sl], Alu.subtract,
                    )
                    nc.vector.tensor_tensor(
                        trs[1][:, sl], wr_tiles[1][:, sl],
                        h8[0][:, 1, sl], Alu.subtract,
                    )
                for sl in S:
                    nc.gpsimd.tensor_tensor(
                        l8[0][:, 0, sl], trs[0][:, sl], m8[0][:, 0, sl],
                        Alu.subtract,
                    )
                for sl in S:
                    nc.vector.tensor_tensor(
                        l8[0][:, 1, sl], trs[1][:, sl], m8[0][:, 1, sl],
                        Alu.subtract,
                    )

            def prep_k(k):
                p, j = divmod(k, 2)
                xk = xu[k]
                if j == 0:
                    alloc_pair(p)
                wr = wr_tiles[k]
                tr = tr_pool.tile([P, O_PER], f32, name="tr", tag="tr")
                # E = exp(W); H/M/L = successive e4m3 roundings.
                # Engine balance (per k, PE pace is ~1284ns/k):
                #   ACT exp+xb1-mid ~1255, GPS H/tr/L ~1281, DVE M+xb1 ~1020
                nc.scalar.activation(wr[:], wr[:], Act.Exp)
                nc.gpsimd.tensor_copy(h8[p][:, j, :], wr[:])
                nc.vector.tensor_tensor(
                    m8[p][:, j, :], wr[:], h8[p][:, j, :], Alu.subtract
                )
                nc.gpsimd.tensor_tensor(
                    tr[:], wr[:], h8[p][:, j, :], Alu.subtract
                )
                l8_eng = nc.gpsimd if l8_gps else nc.vector
                l8_eng.tensor_tensor(
                    l8[p][:, j, :], tr[:], m8[p][:, j, :], Alu.subtract
                )
                # xb = 2x-1 in {-1,+1}, exact in e4m3; phase-1 half only
                if xb1_mode == "d":
                    nc.vector.tensor_scalar(
                        xb[p][:, j, 0:BH], xk[:, 0:BH],
                        2.0, 1.0, Alu.mult, Alu.subtract,
                    )
                else:
                    nc.scalar.activation(
                        xb[p][:, j, 0:546], xk[:, 0:546],
                        Act.Identity, bias=-1.0, scale=2.0,
                    )
                    oth = nc.gpsimd if xb1_mode == "ag" else nc.vector
                    oth.tensor_scalar(
                        xb[p][:, j, 546:BH], xk[:, 546:BH],
                        2.0, 1.0, Alu.mult, Alu.subtract,
                    )

            def mm3(p, m, ps):
                for lvl, tt in enumerate((h8[p], m8[p], l8[p])):
                    nc.tensor.matmul(
                        ps[:],
                        xb[p][:, :, m * P : (m + 1) * P],
                        tt[:, :, :],
                        start=(p == 0 and lvl == 0),
                        stop=(p == PAIRS - 1 and lvl == 2),
                        perf_mode=DR,
                    )

            def evict(m, ps, eng=None):
                otm = ot_pool.tile([P, NFREE], u8, name="otm", tag="otm")
                nc.vector.tensor_scalar(otm[:], ps[:], 0.0, None, Alu.is_gt)
                # phase 1 stores go on SP (ACT is busy with xb2 casts then);
                # phase 2 alternates queues so the tail isn't serialized
                if eng is None:
                    eng = nc.sync if m % 2 == 0 else nc.scalar
                eng.dma_start(out[m * P : (m + 1) * P, :], otm[:])

            # ---- phase 1: m 0..8, k-streaming order ----
            # DMAs run one pair ahead of prep so the Pool queue's in-order
            # compute waits never delay the next pair's transfers.
            pss = {
                m: ps_pool.tile([P, NFREE], f32, name=f"pn{m % MGRP}",
                                tag=f"pn{m % MGRP}")
                for m in range(MGRP)
            }
            dma_k(0)
            dma_k(1)
            # warm the Exp table only after the k1 DMAs are queued on ACT —
            # the 1.3us table load must not delay those issues
            nc.scalar.activation(warm[:], warm[:], Act.Exp)
            for p in range(PAIRS):
                if p + 1 < PAIRS:
                    dma_k(2 * p + 2)
                    dma_k(2 * p + 3)
                if p == 0:
                    prep_pair0()
                else:
                    prep_k(2 * p)
                    prep_k(2 * p + 1)
                # level-outer: the pair's H matmuls only need the H tiles,
                # giving the M/L prep ops ~0.9us of PE time to land
                for lvl, tt in enumerate((h8[p], m8[p], l8[p])):
                    for m in range(MGRP):
                        nc.tensor.matmul(
                            pss[m][:],
                            xb[p][:, :, m * P : (m + 1) * P],
                            tt[:, :, :],
                            start=(p == 0 and lvl == 0),
                            stop=(p == PAIRS - 1 and lvl == 2),
                            perf_mode=DR,
                        )
            # deferred x halves: DMA + casts trail into phase 2.  Emitted
            # BEFORE the phase-1 evicts so the first casts aren't queued
            # behind evict work on DVE/SP; k0/k1 go on ACT which frees first.
            for k in range(KT):
                nc.sync.dma_start(
                    xu[k][:, BH:B_PER], xt[k * P : (k + 1) * P, BH:B_PER]
                )
            def xb2(k):
                p, j = divmod(k, 2)
                # ACT takes even k, GPS odd — DVE stays free for evicts
                if k % 2 == 0:
                    nc.scalar.activation(
                        xb[p][:, j, BH:B_PER], xu[k][:, BH:B_PER],
                        Act.Identity, bias=-1.0, scale=2.0,
                    )
                else:
                    nc.gpsimd.tensor_scalar(
                        xb[p][:, j, BH:B_PER], xu[k][:, BH:B_PER],
                        2.0, 1.0, Alu.mult, Alu.subtract,
                    )

            for k in range(KT):
                xb2(k)
            for m in range(MGRP):
                evict(m, pss[m], eng=nc.sync)

            # ---- phase 2: m 8..16, PE-dense on resident tiles ----
            # m-subgroups so psum chains stop (and evict+store) staggered
            # through the phase instead of piling up at the kernel tail
            pss2 = {
                m: ps_pool.tile([P, NFREE], f32, name=f"pn{m % MGRP}",
                                tag=f"pn{m % MGRP}")
                for m in range(MGRP, MT)
            }
            for sub in ((8, 12), (12, 15), (15, 16)):
                for p in range(PAIRS):
                    for m in range(*sub):
                        mm3(p, m, pss2[m])
                        if p == PAIRS - 1:
                            evict(m, pss2[m])

    if split_waits:
        _split_multi_waits(nc)
    return nc


def _get_compiled():
    if "fp83" not in _COMPILED:
        _COMPILED["fp83"] = _build()
    return _COMPILED["fp83"]


def kernel(x: np.ndarray, raw_weight: np.ndarray, _trace: bool = False):
    from concourse.bass_utils import run_bass_kernel_spmd

    nc = _get_compiled()

    x = np.asarray(x)
    raw_weight = np.asarray(raw_weight)

    # x is exactly 0.0/1.0; uint8 encodes it losslessly and quarters the DMA
    xT = np.ascontiguousarray(x.T.astype(np.uint8))
    wT = np.ascontiguousarray(raw_weight.T).astype(np.float32, copy=False)

    in_maps = []
    for c in range(N_CORES):
        bg, og = divmod(c, OG)
        in_maps.append(
            {
                "xt": np.ascontiguousarray(xT[:, bg * B_PER : (bg + 1) * B_PER]),
                "wt": np.ascontiguousarray(wT[:, og * O_PER : (og + 1) * O_PER]),
            }
        )

    res = run_bass_kernel_spmd(
        nc, in_maps, core_ids=list(range(N_CORES)), trace=_trace
    )

    full = np.empty((BATCH, OUT_F), dtype=x.dtype)
    for c in range(N_CORES):
        bg, og = divmod(c, OG)
        full[bg * B_PER : (bg + 1) * B_PER, og * O_PER : (og + 1) * O_PER] = (
            res.results[c]["out"]
        )
    if _trace:
        kernel.last_results = res
    return full
